# revision 32
# baseline (speedup 1.0000x reference)
import os
import sys

import numpy as np

sys.path.insert(0, "/opt/trn_rl_repo")

import concourse.bass as bass
import concourse.tile as tile
from concourse import bacc, mybir
from concourse.bass_utils import run_bass_kernel_spmd

AF = mybir.ActivationFunctionType
ALU = mybir.AluOpType
AX = mybir.AxisListType
F32 = mybir.dt.float32
BF16 = mybir.dt.bfloat16

CIN, COUT, NG, KH, KW, SD = 256, 128, 4, 3, 3, 512
B, H, W0 = 8, 64, 64
GS = CIN // NG
NTOT = GS * CIN
NSH = NTOT // 8
KDW = KH * KW * SD
HP = H + 2
ROWCH = 20480


def build_kernel():
    nc = bacc.Bacc(num_devices=8)

    xin = nc.declare_dram_parameter("xin", [CIN, H, W0], F32, isOutput=False)
    pwpad = nc.declare_dram_parameter("pwpad", [SD, B * 25], F32, isOutput=False)
    wpool = nc.declare_dram_parameter("wpool", [SD, 1], F32, isOutput=False)
    dw_w = nc.declare_dram_parameter("dw_w", [KDW, NSH], F32, isOutput=False)
    dw_b = nc.declare_dram_parameter("dw_b", [1, NSH], F32, isOutput=False)
    pw_w = nc.declare_dram_parameter("pw_w", [SD, NSH], F32, isOutput=False)
    pw_b = nc.declare_dram_parameter("pw_b", [1, NSH], F32, isOutput=False)
    bias_w = nc.declare_dram_parameter("bias_w", [SD, CIN], F32, isOutput=False)
    bias_b = nc.declare_dram_parameter("bias_b", [CIN, 1], F32, isOutput=False)
    ada_w = nc.declare_dram_parameter("ada_w", [9, CIN, CIN], F32, isOutput=False)
    ada_b = nc.declare_dram_parameter("ada_b", [CIN, 1], F32, isOutput=False)
    c0_w = nc.declare_dram_parameter("c0_w", [9, CIN, CIN], F32, isOutput=False)
    c0_b = nc.declare_dram_parameter("c0_b", [CIN, 1], F32, isOutput=False)
    c1_w = nc.declare_dram_parameter("c1_w", [9, CIN, COUT], F32, isOutput=False)
    c1_b = nc.declare_dram_parameter("c1_b", [COUT, 1], F32, isOutput=False)
    out = nc.declare_dram_parameter("out", [COUT, 2 * H, 2 * W0], F32, isOutput=True)

    dbg = os.environ.get("BASS_DEBUG_OUT") == "1"
    dbgt = {}
    if dbg:
        dbgt["xn"] = nc.declare_dram_parameter("dbg_xn", [128, HP * HP], BF16, isOutput=True)
        dbgt["send"] = nc.declare_dram_parameter("dbg_send", [8, ROWCH], BF16, isOutput=True)
        dbgt["recv"] = nc.declare_dram_parameter("dbg_recv", [8, ROWCH], BF16, isOutput=True)
        dbgt["dkq"] = nc.declare_dram_parameter("dbg_dkq", [128, 9 * 128], BF16, isOutput=True)
        dbgt["y2"] = nc.declare_dram_parameter("dbg_y2", [128, HP * HP], BF16, isOutput=True)
        dbgt["ada"] = nc.declare_dram_parameter("dbg_ada", [128, HP * HP], BF16, isOutput=True)
        dbgt["c0"] = nc.declare_dram_parameter("dbg_c0", [128, HP * HP], BF16, isOutput=True)
        dbgt["yw"] = nc.declare_dram_parameter("dbg_yw", [128, H * 128], F32, isOutput=True)

    send = nc.dram_tensor("send_buf", [8, ROWCH], BF16, kind="Internal")
    recv = nc.dram_tensor("recv_buf", [8, ROWCH], BF16, kind="Internal")

    with tile.TileContext(nc) as tc, \
         tc.tile_pool(name="singles", bufs=1) as singles, \
         tc.tile_pool(name="pad", bufs=4) as padp, \
         tc.tile_pool(name="fr4k", bufs=2) as fr4k, \
         tc.tile_pool(name="dw", bufs=3) as dwp, \
         tc.tile_pool(name="dwb", bufs=3) as dwbp, \
         tc.tile_pool(name="wts", bufs=1) as wtsp, \
         tc.tile_pool(name="s512", bufs=6) as s512, \
         tc.tile_pool(name="s512b", bufs=4) as s512b, \
         tc.tile_pool(name="up", bufs=2) as upp, \
         tc.tile_pool(name="small", bufs=16) as smallp, \
         tc.tile_pool(name="ps", bufs=8, space="PSUM") as psp:

        pwsbb = []
        for ct in range(4):
            t = singles.tile([128, B * 25], F32, tag=f"pwsbf{ct}", name=f"pwsf{ct}")
            nc.sync.dma_start(out=t[:], in_=pwpad[128 * ct:128 * (ct + 1), :])
            tb = singles.tile([128, B * 25], BF16, tag=f"pwsbb{ct}", name=f"pwsb{ct}")
            nc.vector.tensor_copy(out=tb[:], in_=t[:])
            pwsbb.append(tb)

        wpall = []
        for ct in range(4):
            tf = pwsbb[ct]
            pw3 = tf[:].rearrange("p (b o) -> p b o", o=25)
            t = smallp.tile([128, B], F32, tag="wpall", name=f"wpa{ct}")
            nc.vector.tensor_tensor(out=t[:], in0=pw3[:, :, 6], in1=pw3[:, :, 7],
                                    op=ALU.add)
            t2_ = smallp.tile([128, B], F32, tag="wpall", name=f"wpb{ct}")
            nc.vector.tensor_tensor(out=t2_[:], in0=pw3[:, :, 11],
                                    in1=pw3[:, :, 12], op=ALU.add)
            nc.vector.tensor_tensor(out=t[:], in0=t[:], in1=t2_[:], op=ALU.add)
            tb = smallp.tile([128, B], BF16, tag="wpallb", name=f"wpc{ct}")
            nc.vector.tensor_scalar(out=tb[:], in0=t[:], scalar1=0.25,
                                    scalar2=None, op0=ALU.mult)
            wpall.append(tb)

        xp = []
        for ct in range(2):
            xs = fr4k.tile([128, H * W0], F32, tag="fr4k", name=f"xs{ct}")
            nc.sync.dma_start(out=xs[:], in_=xin[128 * ct:128 * (ct + 1), :, :])
            ssum = smallp.tile([128, 1], F32, tag="st", name=f"ssum{ct}")
            nc.vector.reduce_sum(out=ssum[:], in_=xs[:], axis=AX.X)
            sq = fr4k.tile([128, H * W0], F32, tag="fr4k", name=f"sq{ct}")
            sqs = smallp.tile([128, 1], F32, tag="st", name=f"sqs{ct}")
            nc.scalar.activation(out=sq[:], in_=xs[:], func=AF.Square,
                                 accum_out=sqs[:])
            inv_n = 1.0 / (H * W0)
            negmean = smallp.tile([128, 1], F32, tag="st", name=f"nm{ct}")
            nc.vector.tensor_scalar(out=negmean[:], in0=ssum[:], scalar1=-inv_n,
                                    scalar2=None, op0=ALU.mult)
            mean = smallp.tile([128, 1], F32, tag="st", name=f"mn{ct}")
            nc.vector.tensor_scalar(out=mean[:], in0=ssum[:], scalar1=inv_n,
                                    scalar2=None, op0=ALU.mult)
            ex2 = smallp.tile([128, 1], F32, tag="st", name=f"ex{ct}")
            nc.vector.tensor_scalar(out=ex2[:], in0=sqs[:], scalar1=inv_n,
                                    scalar2=None, op0=ALU.mult)
            m2 = smallp.tile([128, 1], F32, tag="st", name=f"m2{ct}")
            nc.vector.tensor_tensor(out=m2[:], in0=mean[:], in1=mean[:],
                                    op=ALU.mult)
            var = smallp.tile([128, 1], F32, tag="st", name=f"vr{ct}")
            nc.vector.tensor_tensor(out=var[:], in0=ex2[:], in1=m2[:],
                                    op=ALU.subtract)
            epsb = smallp.tile([128, 1], F32, tag="st", name=f"ep{ct}")
            nc.vector.memset(epsb[:], 0.001)
            std = smallp.tile([128, 1], F32, tag="st", name=f"sd{ct}")
            nc.scalar.activation(out=std[:], in_=var[:], func=AF.Sqrt,
                                 bias=epsb[:])
            rstd = smallp.tile([128, 1], F32, tag="st", name=f"rs{ct}")
            nc.vector.reciprocal(out=rstd[:], in_=std[:])

            t = padp.tile([128, HP * HP], BF16, tag="pad", name=f"xp{ct}")
            t3 = t[:].rearrange("p (r c) -> p r c", c=HP)
            xs3 = xs[:].rearrange("p (r c) -> p r c", c=W0)
            nc.vector.tensor_scalar(out=t3[:, 1:1 + H, 1:1 + W0], in0=xs3,
                                    scalar1=negmean[:], scalar2=rstd[:],
                                    op0=ALU.add, op1=ALU.mult)
            _reflect_borders(nc, t3)
            if dbg and ct == 0:
                nc.sync.dma_start(out=dbgt["xn"][:], in_=t[:])
            xp.append(t)

        wpsb = []
        for kt in range(4):
            tf = smallp.tile([128, 1], F32, tag="st", name=f"wpf{kt}")
            nc.sync.dma_start(out=tf[:], in_=wpool[128 * kt:128 * (kt + 1), :])
            t = smallp.tile([128, 1], BF16, tag="wp", name=f"wpq{kt}")
            nc.vector.tensor_copy(out=t[:], in_=tf[:])
            wpsb.append(t)
        badd = []
        for t2 in range(2):
            ps = psp.tile([128, 512], F32, tag="ps", name=f"bps{t2}")
            for kt in range(4):
                lwf = smallp.tile([128, 128], F32, tag="bw", name=f"lwf{t2}{kt}",
                                  bufs=4)
                nc.sync.dma_start(
                    out=lwf[:],
                    in_=bias_w[128 * kt:128 * (kt + 1), 128 * t2:128 * (t2 + 1)])
                lw = smallp.tile([128, 128], BF16, tag="bwb", name=f"lwb{t2}{kt}",
                                 bufs=4)
                nc.vector.tensor_copy(out=lw[:], in_=lwf[:])
                nc.tensor.matmul(ps[:, 0:1], lw[:], wpsb[kt][:],
                                 start=(kt == 0), stop=(kt == 3))
            bb = smallp.tile([128, 1], F32, tag="st", name=f"bb{t2}")
            nc.sync.dma_start(out=bb[:], in_=bias_b[128 * t2:128 * (t2 + 1), :])
            bs = smallp.tile([128, 1], F32, tag="badd", name=f"bs{t2}")
            nc.vector.tensor_tensor(out=bs[:], in0=ps[:, 0:1], in1=bb[:], op=ALU.add)
            badd.append(bs)

        ones72 = singles.tile([128, 72], BF16, tag="ones")
        nc.vector.memset(ones72[:], 1.0)

        patches = []
        for kt in range(36):
            pos, ct = kt // 4, kt % 4
            ri, rj = pos // 3, pos % 3
            pw3 = pwsbb[ct][:].rearrange("p (b r c) -> p b r c", r=5, c=5)
            pt = singles.tile([128, 72], BF16, tag=f"patch{kt}", name=f"pat{kt}")
            pt3 = pt[:].rearrange("p (b r c) -> p b r c", r=3, c=3)
            nc.vector.tensor_copy(out=pt3, in_=pw3[:, :, ri:ri + 3, rj:rj + 3])
            patches.append(pt)

        dwps = [psp.tile([128, 512], F32, tag="ps", name=f"dwps{i}")
                for i in range(4)]
        for kt in range(36):
            dwt = dwp.tile([128, NSH], F32, tag="dw", name="dwt")
            nc.sync.dma_start(out=dwt[:], in_=dw_w[128 * kt:128 * (kt + 1), :])
            dwtb = dwbp.tile([128, NSH], BF16, tag="dwb", name="dwtb")
            nc.scalar.activation(out=dwtb[:, 0:1024], in_=dwt[:, 0:1024],
                                 func=AF.Copy)
            nc.vector.tensor_copy(out=dwtb[:, 1024:2048], in_=dwt[:, 1024:2048])
            for nt in range(4):
                nc.tensor.matmul(dwps[nt][0:72, :],
                                 patches[kt][:], dwtb[:, 512 * nt:512 * (nt + 1)],
                                 start=(kt == 0), stop=False)
        for nt in range(4):
            dwbf = s512.tile([128, 512], F32, tag="s512", name="dwbf")
            nc.sync.dma_start(out=dwbf[0:1, :], in_=dw_b[:, 512 * nt:512 * (nt + 1)])
            dwbb = s512b.tile([128, 512], BF16, tag="s512b", name="dwbb")
            nc.vector.tensor_copy(out=dwbb[0:1, :], in_=dwbf[0:1, :])
            nc.tensor.matmul(dwps[nt][0:72, :], ones72[0:1, :],
                             dwbb[0:1, :], start=False, stop=True)

        send3 = send[:].rearrange("b (k n) -> b k n", n=NSH)
        for nt in range(4):
            dws = s512b.tile([128, 512], BF16, tag="s512b", name="dws")
            nc.vector.tensor_copy(out=dws[0:72, :], in_=dwps[nt][0:72, :])
            nc.sync.dma_start(out=send3[:, 0:9, 512 * nt:512 * (nt + 1)],
                              in_=dws[0:72, :])

        pwps = [psp.tile([128, 512], F32, tag="ps", name=f"pwps{i}")
                for i in range(4)]
        for kt in range(4):
            pwt = dwp.tile([128, NSH], F32, tag="dw", name="pwt")
            nc.sync.dma_start(out=pwt[:], in_=pw_w[128 * kt:128 * (kt + 1), :])
            pwtb = dwbp.tile([128, NSH], BF16, tag="dwb", name="pwtb")
            nc.scalar.activation(out=pwtb[:, 0:1024], in_=pwt[:, 0:1024],
                                 func=AF.Copy)
            nc.vector.tensor_copy(out=pwtb[:, 1024:2048], in_=pwt[:, 1024:2048])
            for nt in range(4):
                nc.tensor.matmul(pwps[nt][0:8, :], wpall[kt][:],
                                 pwtb[:, 512 * nt:512 * (nt + 1)],
                                 start=(kt == 0), stop=False)
        for nt in range(4):
            pwbf = s512.tile([128, 512], F32, tag="s512", name="pwbf")
            nc.sync.dma_start(out=pwbf[0:1, :], in_=pw_b[:, 512 * nt:512 * (nt + 1)])
            pwbb = s512b.tile([128, 512], BF16, tag="s512b", name="pwbb")
            nc.vector.tensor_copy(out=pwbb[0:1, :], in_=pwbf[0:1, :])
            nc.tensor.matmul(pwps[nt][0:8, :], ones72[0:1, 0:8],
                             pwbb[0:1, :], start=False, stop=True)
        for nt in range(4):
            pws = s512b.tile([128, 512], BF16, tag="s512b", name="pws")
            nc.vector.tensor_copy(out=pws[0:8, :], in_=pwps[nt][0:8, :])
            nc.sync.dma_start(out=send3[:, 9, 512 * nt:512 * (nt + 1)],
                              in_=pws[0:8, :])

        if dbg:
            nc.sync.dma_start(out=dbgt["send"][:], in_=send[:])

        nc.gpsimd.collective_compute(
            "AllToAll", ALU.bypass, replica_groups=[list(range(8))],
            ins=[send[:]], outs=[recv[:]])
        if dbg:
            nc.sync.dma_start(out=dbgt["recv"][:], in_=recv[:])

        warm_scratch = nc.dram_tensor("warm_scratch", [128, 512], F32,
                                      kind="Internal")
        wps = psp.tile([128, 512], F32, tag="ps", name="warmps")
        for wi in range(240):
            nc.tensor.matmul(wps[0:72, 0:200], patches[wi % 36][:],
                             pwsbb[wi % 4][:],
                             start=(wi == 0), stop=(wi == 239))
        wcp = s512.tile([128, 512], F32, tag="s512", name="wcp")
        nc.vector.tensor_copy(out=wcp[:], in_=wps[:])
        nc.sync.dma_start(out=warm_scratch[:], in_=wcp[:])

        adasb = _load_conv_w(nc, wtsp, smallp, ada_w, 2, 2, "wA")
        c0sb = _load_conv_w(nc, wtsp, smallp, c0_w, 2, 2, "wB")
        adabs = _load_bias(nc, smallp, ada_b, 2, "ab")
        c0bs = _load_bias(nc, smallp, c0_b, 2, "cb")
        c1bs = _load_bias(nc, smallp, c1_b, 1, "db")

        recv4 = recv[:].rearrange("r (k iw o) -> r k iw o", k=10, o=CIN)
        dkq, pkq = [], []
        for t2 in range(2):
            dk = singles.tile([128, 9 * 128], BF16, tag=f"dkq{t2}", name=f"dkq{t2}")
            nc.vector.memset(dk[:], 0.0)
            for k in range(9):
                for g in range(2):
                    nc.sync.dma_start(
                        out=dk[64 * g:64 * (g + 1),
                               128 * k + 64 * g:128 * k + 64 * (g + 1)],
                        in_=recv4[:, k, :, 128 * t2 + 64 * g:128 * t2 + 64 * (g + 1)])
            dkq.append(dk)
            pk = singles.tile([128, 128], BF16, tag=f"pkq{t2}", name=f"pkq{t2}")
            nc.vector.memset(pk[:], 0.0)
            for g in range(2):
                nc.sync.dma_start(
                    out=pk[64 * g:64 * (g + 1), 64 * g:64 * (g + 1)],
                    in_=recv4[:, 9, :, 128 * t2 + 64 * g:128 * t2 + 64 * (g + 1)])
            pkq.append(pk)
        if dbg:
            nc.sync.dma_start(out=dbgt["dkq"][:], in_=dkq[0][:])

        y2p = []
        for t2 in range(2):
            t = padp.tile([128, HP * HP], BF16, tag="pad", name=f"y2p{t2}")
            y2p.append(t)
        for t2 in range(2):
            xp3 = xp[t2][:].rearrange("p (r c) -> p r c", c=HP)
            o3 = y2p[t2][:].rearrange("p (r c) -> p r c", c=HP)
            ps1s = [psp.tile([128, 512], F32, tag="ps", name=f"ps1_{t2}{r}")
                    for r in range(8)]
            for k in range(9):
                di, dj = k // 3, k % 3
                for rg in range(8):
                    rhs = xp3[:, rg * 8 + di:rg * 8 + di + 8, dj:dj + W0]
                    nc.tensor.matmul(ps1s[rg][:],
                                     dkq[t2][:, 128 * k:128 * (k + 1)],
                                     rhs, start=(k == 0), stop=(k == 8))
            for rg in range(8):
                y1s = s512b.tile([128, 512], BF16, tag="s512b", name=f"y1s{rg}")
                nc.vector.tensor_copy(out=y1s[:], in_=ps1s[rg][:])
                ps2 = psp.tile([128, 512], F32, tag="ps", name="ps2")
                nc.tensor.matmul(ps2[:], pkq[t2][:], y1s[:],
                                 start=True, stop=True)
                dst = o3[:, 1 + rg * 8:1 + rg * 8 + 8, 1:1 + W0]
                src = ps2[:].rearrange("p (r c) -> p r c", c=W0)
                nc.vector.tensor_scalar(out=dst, in0=src, scalar1=badd[t2][:],
                                        scalar2=None, op0=ALU.add)
            _reflect_borders(nc, o3)
        if dbg:
            nc.sync.dma_start(out=dbgt["y2"][:], in_=y2p[0][:])

        adap = _conv3x3(nc, padp, psp, y2p, adasb, adabs, lrelu=False)
        if dbg:
            nc.sync.dma_start(out=dbgt["ada"][:], in_=adap[0][:])
        c1sb = _load_conv_w(nc, wtsp, smallp, c1_w, 2, 1, "wA")
        c0p = _conv3x3(nc, padp, psp, adap, c0sb, c0bs, lrelu=True)
        if dbg:
            nc.sync.dma_start(out=dbgt["c0"][:], in_=c0p[0][:])

        yw = singles.tile([128, H * 128], F32, tag="yw")
        yw3 = yw[:].rearrange("p (r c) -> p r c", c=128)
        c0p3 = [c0p[kt][:].rearrange("p (r c) -> p r c", c=HP) for kt in range(2)]

        c1ps = [psp.tile([128, 512], F32, tag="ps", name=f"c1ps{r}")
                for r in range(8)]

        def conv1_all():
            for kt in range(2):
                for k in range(9):
                    di, dj = k // 3, k % 3
                    for rg in range(8):
                        rhs = c0p3[kt][:, rg * 8 + di:rg * 8 + di + 8, dj:dj + W0]
                        nc.tensor.matmul(c1ps[rg][:], c1sb[kt][k][0][:], rhs,
                                         start=(kt == 0 and k == 0),
                                         stop=(kt == 1 and k == 8))

        def conv1_epi(rg):
            y4 = s512.tile([128, 512], F32, tag="s512", name="y4")
            nc.scalar.activation(out=y4[:], in_=c1ps[rg][:], func=AF.Prelu,
                                 bias=c1bs[0][:], scale=1.0, alpha=0.2)
            return y4

        def wpass_rg(rg, y4):
            y43 = y4[:].rearrange("p (r c) -> p r c", c=W0)
            a = s512.tile([128, 512], F32, tag="s512", name="upa")
            nc.scalar.activation(out=a[:], in_=y4[:], func=AF.Copy, scale=0.75)
            a3 = a[:].rearrange("p (r c) -> p r c", c=W0)
            b_ = s512.tile([128, 512], F32, tag="s512", name="upb")
            nc.vector.tensor_scalar(out=b_[:], in0=y4[:], scalar1=0.25,
                                    scalar2=None, op0=ALU.mult)
            b3 = b_[:].rearrange("p (r c) -> p r c", c=W0)
            rows = yw3[:, rg * 8:rg * 8 + 8, :]
            nc.vector.tensor_copy(out=rows[:, :, 0], in_=y43[:, :, 0])
            nc.vector.tensor_copy(out=rows[:, :, 127], in_=y43[:, :, 63])
            nc.vector.tensor_tensor(out=rows[:, :, 2:127:2], in0=a3[:, :, 1:64],
                                    in1=b3[:, :, 0:63], op=ALU.add)
            nc.vector.tensor_tensor(out=rows[:, :, 1:127:2], in0=a3[:, :, 0:63],
                                    in1=b3[:, :, 1:64], op=ALU.add)

        out3 = out[:]

        def hpass_block(hb):
            klo = 4 * hb
            yu = upp.tile([128, 8 * 128], F32, tag="yu", name="yu")
            yu3 = yu[:].rearrange("p (r c) -> p r c", c=128)
            blo = max(klo - 1, 0)
            bhi = min(klo + 5, H)
            tb = upp.tile([128, 6 * 128], F32, tag="tb", name="tb")
            tb3 = tb[:].rearrange("p (r c) -> p r c", c=128)
            nb = bhi - blo
            nc.vector.tensor_scalar(out=tb3[:, 0:nb, :], in0=yw3[:, blo:bhi, :],
                                    scalar1=0.25, scalar2=None, op0=ALU.mult)
            for par in range(2):
                j0 = 0
                if hb == 0 and par == 0:
                    nc.vector.tensor_copy(out=yu3[:, 0, :], in_=yw3[:, 0, :])
                    j0 = 1
                jn = 4
                if hb == 15 and par == 1:
                    nc.vector.tensor_copy(out=yu3[:, 7, :], in_=yw3[:, 63, :])
                    jn = 3
                if j0 >= jn:
                    continue
                kk = klo + j0
                n = jn - j0
                dst = yu3[:, 2 * j0 + par:2 * (jn - 1) + par + 1:2, :]
                nc.scalar.activation(out=dst, in_=yw3[:, kk:kk + n, :],
                                     func=AF.Copy, scale=0.75)
                src1 = tb3[:, kk + (2 * par - 1) - blo:kk + (2 * par - 1) - blo + n, :]
                nc.vector.tensor_tensor(out=dst, in0=dst, in1=src1, op=ALU.add)
            nc.sync.dma_start(out=out3[:, 8 * hb:8 * hb + 8, :], in_=yu[:])

        conv1_all()
        for rg in range(8):
            y4 = conv1_epi(rg)
            wpass_rg(rg, y4)
            for hb in range(max(0, 2 * rg - 2), 2 * rg):
                hpass_block(hb)
        hpass_block(14)
        hpass_block(15)
        if dbg:
            nc.sync.dma_start(out=dbgt["yw"][:], in_=yw[:])

    nc.compile()
    return nc


def _reflect_borders(nc, t3):
    nc.vector.tensor_copy(out=t3[:, 0, 1:65], in_=t3[:, 2, 1:65])
    nc.vector.tensor_copy(out=t3[:, 65, 1:65], in_=t3[:, 63, 1:65])
    nc.vector.tensor_copy(out=t3[:, :, 0], in_=t3[:, :, 2])
    nc.vector.tensor_copy(out=t3[:, :, 65], in_=t3[:, :, 63])


def _load_conv_w(nc, pool, stagep, w, nkt, nt2, tagpfx):
    sb = []
    i = 0
    for kt in range(nkt):
        per_k = []
        for k in range(9):
            per_t2 = []
            for t2 in range(nt2):
                tf = stagep.tile([128, 128], F32, tag="bw", name=f"{tagpfx}f{i}",
                                 bufs=4)
                nc.sync.dma_start(
                    out=tf[:],
                    in_=w[k, 128 * kt:128 * (kt + 1), 128 * t2:128 * (t2 + 1)])
                t = pool.tile([128, 128], BF16, tag=f"{tagpfx}_{i}",
                              name=f"{tagpfx}b{i}")
                nc.vector.tensor_copy(out=t[:], in_=tf[:])
                i += 1
                per_t2.append(t)
            per_k.append(per_t2)
        sb.append(per_k)
    return sb


def _load_bias(nc, pool, bvec, nt2, tagpfx):
    bs = []
    for t2 in range(nt2):
        t = pool.tile([128, 1], F32, tag="cbias", name=f"{tagpfx}{t2}")
        nc.sync.dma_start(out=t[:], in_=bvec[128 * t2:128 * (t2 + 1), :])
        bs.append(t)
    return bs


def _conv3x3(nc, padp, psp, src_p, wsb, bsb, lrelu):
    src3 = [src_p[kt][:].rearrange("p (r c) -> p r c", c=HP) for kt in range(2)]
    dst = []
    for t2 in range(2):
        t = padp.tile([128, HP * HP], BF16, tag="pad", name=f"cv{t2}")
        dst.append(t)
    for t2 in range(2):
        o3 = dst[t2][:].rearrange("p (r c) -> p r c", c=HP)
        pss = [psp.tile([128, 512], F32, tag="ps", name=f"cvps{t2}{r}")
               for r in range(8)]
        for kt in range(2):
            for k in range(9):
                di, dj = k // 3, k % 3
                for rg in range(8):
                    rhs = src3[kt][:, rg * 8 + di:rg * 8 + di + 8, dj:dj + W0]
                    nc.tensor.matmul(pss[rg][:], wsb[kt][k][t2][:], rhs,
                                     start=(kt == 0 and k == 0),
                                     stop=(kt == 1 and k == 8))
        for rg in range(8):
            d = o3[:, 1 + rg * 8:1 + rg * 8 + 8, 1:1 + W0]
            if lrelu:
                nc.scalar.activation(out=d, in_=pss[rg][:].rearrange(
                    "p (r c) -> p r c", c=W0), func=AF.Prelu,
                    bias=bsb[t2][:], scale=1.0, alpha=0.2)
            else:
                nc.vector.tensor_scalar(out=d, in0=pss[rg][:].rearrange(
                    "p (r c) -> p r c", c=W0), scalar1=bsb[t2][:],
                    scalar2=None, op0=ALU.add)
        _reflect_borders(nc, o3)
    return dst


_NC_CACHE = {}


def _get_nc():
    if "nc" not in _NC_CACHE:
        _NC_CACHE["nc"] = build_kernel()
    return _NC_CACHE["nc"]


def kernel(x, W, dw_pred_w, dw_pred_b, pw_pred_w, pw_pred_b,
           bias_pred_w, bias_pred_b, ada_conv_w, ada_conv_b,
           conv0_w, conv0_b, conv1_w, conv1_b, _trace=False,
           _return_res=False):
    x = np.asarray(x, np.float32)
    W = np.asarray(W, np.float32)

    pW = np.pad(W, ((0, 0), (1, 1), (1, 1), (0, 0)), mode="reflect")
    pwpad = np.ascontiguousarray(pW.transpose(3, 0, 1, 2)).reshape(SD, B * 25)
    wpool = W[:, :2, :2, :].mean(axis=(1, 2)).astype(np.float32)
    dw2 = np.asarray(dw_pred_w, np.float32).reshape(KDW, NTOT)
    pw2 = np.asarray(pw_pred_w, np.float32)
    ada_r = np.ascontiguousarray(np.asarray(ada_conv_w, np.float32).reshape(9, CIN, CIN))
    c0_r = np.ascontiguousarray(np.asarray(conv0_w, np.float32).reshape(9, CIN, CIN))
    c1_r = np.ascontiguousarray(np.asarray(conv1_w, np.float32).reshape(9, CIN, COUT))
    rep = dict(
        pwpad=pwpad,
        bias_w=np.ascontiguousarray(np.asarray(bias_pred_w, np.float32)),
        bias_b=np.asarray(bias_pred_b, np.float32).reshape(CIN, 1),
        ada_w=ada_r, ada_b=np.asarray(ada_conv_b, np.float32).reshape(CIN, 1),
        c0_w=c0_r, c0_b=np.asarray(conv0_b, np.float32).reshape(CIN, 1),
        c1_w=c1_r, c1_b=np.asarray(conv1_b, np.float32).reshape(COUT, 1),
    )
    in_maps = []
    for c in range(8):
        m = dict(rep)
        m["xin"] = np.ascontiguousarray(x[c].transpose(2, 0, 1))
        m["wpool"] = np.ascontiguousarray(wpool[c].reshape(SD, 1))
        m["dw_w"] = np.ascontiguousarray(dw2[:, NSH * c:NSH * (c + 1)])
        m["dw_b"] = np.asarray(dw_pred_b, np.float32)[NSH * c:NSH * (c + 1)].reshape(1, NSH)
        m["pw_w"] = np.ascontiguousarray(pw2[:, NSH * c:NSH * (c + 1)])
        m["pw_b"] = np.asarray(pw_pred_b, np.float32)[NSH * c:NSH * (c + 1)].reshape(1, NSH)
        in_maps.append(m)

    nc = _get_nc()
    res = run_bass_kernel_spmd(nc, in_maps, core_ids=list(range(8)), trace=_trace)
    outs = [np.moveaxis(res.results[c]["out"], 0, -1) for c in range(8)]
    full = np.stack(outs, axis=0)
    if _trace or _return_res:
        return full, res
    return full
